# revision 21
# baseline (speedup 1.0000x reference)
"""Trainium2 Bass kernel for nn_EncoderBlock (dense transformer block).

Sharding: 8 cores = 4 batches x 2 query-row halves. Each core computes the
full K/V for its batch (duplicated across the pair) and the full pipeline for
its 512 query rows; no collectives. Per-core inputs are rolled along the
sequence axis so every core's query rows are local rows 0..511, letting all
cores run one SPMD program; the host un-rolls the wt columns on gather.

Attention computes the score matrix in BOTH orientations (q-stationary for
the softmax/wt output, k-stationary for the wt^T @ v contraction) which is
cheaper than transposing 4M score elements on-chip. The two streams are
software-pipelined per head-pair so the scalar engine (exp) and tensor
engine (matmuls) overlap and the PE stays HAM-warm.

Outputs: (x_out [4,1024,512], wt [4,8,1024,1024]) matching the reference.
"""

import numpy as np

B, S, DM, H, DK, DV, DF = 4, 1024, 512, 8, 64, 64, 2048
SH = S // 2          # query rows per core
P = 128
EPS = 1e-5
NCORES = 8

# matmul dtype config: "f32r" (fast, ~1e-4 rel err) or "f32" (exact, 4x slower)
MM_MODE = "f32r"

_prog_cache = {}


def _build_program(mm_mode, trivial):
    import concourse.mybir as mybir
    import concourse.tile as tile
    from concourse import bacc
    from concourse.masks import make_identity
    from contextlib import ExitStack

    dt = mybir.dt
    AF = mybir.ActivationFunctionType
    f32 = dt.float32
    mdt = dt.float32r if mm_mode == "f32r" else dt.float32
    KA = DK if trivial else DK + 1   # contraction depth incl. mask-aug row

    nc = bacc.Bacc(None, target_bir_lowering=False)

    # ---- DRAM I/O ----
    xb_d = nc.dram_tensor("xb", [S, DM], f32, kind="ExternalInput")
    qw_d = nc.dram_tensor("qw", [DM, DM], mdt, kind="ExternalInput")
    kw_d = nc.dram_tensor("kw", [DM, DM], mdt, kind="ExternalInput")
    vw_d = nc.dram_tensor("vw", [DM, DM], mdt, kind="ExternalInput")
    wo_d = nc.dram_tensor("wo", [DM, DM], mdt, kind="ExternalInput")
    ffa_d = nc.dram_tensor("ffa", [DM, DF], mdt, kind="ExternalInput")
    ffb_d = nc.dram_tensor("ffb", [DF, DM], mdt, kind="ExternalInput")
    if not trivial:
        madd_d = nc.dram_tensor("madd", [H, S], mdt, kind="ExternalInput")
        bqt_d = nc.dram_tensor("bqt", [DK, H], f32, kind="ExternalInput")
        bkt_d = nc.dram_tensor("bkt", [DK, H], f32, kind="ExternalInput")
        worow_d = nc.dram_tensor("worow", [1, DM], f32, kind="ExternalInput")
        bfa_d = nc.dram_tensor("bfa", [DF], f32, kind="ExternalInput")
        ffbrow_d = nc.dram_tensor("ffbrow", [1, DM], f32, kind="ExternalInput")
    wt_d = nc.dram_tensor("wt_out", [H, SH, S], f32, kind="ExternalOutput")
    xo_d = nc.dram_tensor("x_out", [SH, DM], f32, kind="ExternalOutput")
    rz_dram = nc.dram_tensor("rz_scratch", [P, 32], f32)

    NS = S // P    # 8 t-chunks
    NSH = SH // P  # 4 s-chunks (query half)
    ND = DM // P   # 4 d-chunks
    NF = DF // P   # 16 df-chunks
    NHV = H * DV // P  # 4 chunks of concatenated head outputs

    with tile.TileContext(nc) as tc, ExitStack() as ctx:
        const = ctx.enter_context(tc.tile_pool(name="const", bufs=1))
        big = ctx.enter_context(tc.tile_pool(name="big", bufs=1))
        stats = ctx.enter_context(tc.tile_pool(name="stats", bufs=2))

        idn = const.tile([P, P], f32)
        make_identity(nc, idn[:])
        one_row = const.tile([1, S], f32)
        nc.gpsimd.memset(one_row[:], 1.0)
        eps_t = const.tile([P, 1], f32)
        nc.gpsimd.memset(eps_t[:], EPS)

        # ---- phase A: input ----
        X = big.tile([P, NS, DM], f32)           # x rolled, s-chunk major
        nc.sync.dma_start(X[:], xb_d.rearrange("(c p) d -> p c d", p=P))

        def layernorm_t(src, nchunks, dest_t, tagp, ln_pool, tp_ps):
            """LN over the free dim of src chunks, writing transposed output."""
            for si in range(nchunks):
                st6 = stats.tile([P, 6], f32, tag="st6" + tagp)
                nc.vector.bn_stats(st6[:], src[:, si, :])
                st2 = stats.tile([P, 2], f32, tag="st2" + tagp)
                nc.vector.bn_aggr(st2[:], st6[:])
                std = stats.tile([P, 1], f32, tag="std" + tagp)
                nc.scalar.activation(std[:], st2[:, 1:2], AF.Sqrt, bias=eps_t[:])
                r = stats.tile([P, 1], f32, tag="r" + tagp)
                nc.vector.reciprocal(r[:], std[:])
                nmr = stats.tile([P, 1], f32, tag="nmr" + tagp)
                nc.vector.scalar_tensor_tensor(
                    out=nmr[:], in0=st2[:, 0:1], scalar=-1.0, in1=r[:],
                    op0=mybir.AluOpType.mult, op1=mybir.AluOpType.mult)
                hn = ln_pool.tile([P, DM], f32, tag="hn" + tagp)
                nc.scalar.activation(hn[:], src[:, si, :], AF.Identity,
                                     bias=nmr[:], scale=r[:])
                for dj in range(ND):
                    tp = tp_ps.tile([P, P], f32, tag="tp" + tagp)
                    nc.tensor.transpose(tp[:], hn[:, dj * P:(dj + 1) * P], idn[:])
                    nc.vector.tensor_copy(dest_t[:, dj, si * P:(si + 1) * P],
                                          tp[:])

        # ---- phase B: LN1 -> h1T ----
        h1t = big.tile([P, ND, S], mdt)          # [d-part, d-chunk, s]
        with tc.tile_pool(name="ln", bufs=3) as ln_pool, \
             tc.tile_pool(name="tp_ps", bufs=2, space="PSUM") as tp_ps:
            layernorm_t(X, NS, h1t, "a", ln_pool, tp_ps)

        # ---- phase C: projections qT, kT, v ----
        QT = big.tile([KA, H, SH], mdt)
        KT = big.tile([KA, H, S], mdt)
        V = big.tile([P, NS, DM], mdt)           # [t-part, t-chunk, (h dv)]
        if not trivial:
            nc.gpsimd.memset(QT[DK:DK + 1, :, :].bitcast(f32), 1.0)
            nc.sync.dma_start(KT[DK:DK + 1, :, :], madd_d[None, :, :])

        with tc.tile_pool(name="wpool", bufs=1) as wp, \
             tc.tile_pool(name="proj_ps", bufs=2, space="PSUM") as pps:
            qw = wp.tile([P, ND, DM], mdt, tag="qw")
            kw = wp.tile([P, ND, DM], mdt, tag="kw")
            vw = wp.tile([P, ND, DM], mdt, tag="vw")
            nc.sync.dma_start(qw[:], qw_d.rearrange("(j p) f -> p j f", p=P))
            nc.sync.dma_start(kw[:], kw_d.rearrange("(j p) f -> p j f", p=P))
            nc.sync.dma_start(vw[:], vw_d.rearrange("(j p) f -> p j f", p=P))
            if not trivial:
                bqt = wp.tile([DK, H], f32, tag="bqt")
                bkt = wp.tile([DK, H], f32, tag="bkt")
                nc.sync.dma_start(bqt[:], bqt_d[:, :])
                nc.sync.dma_start(bkt[:], bkt_d[:, :])

            # v (natural layout, all heads wide)
            for ti in range(NS):
                vp = pps.tile([P, DM], f32, tag="vp")
                for dj in range(ND):
                    nc.tensor.matmul(
                        vp[:], h1t[:, dj, ti * P:(ti + 1) * P], vw[:, dj, :],
                        start=(dj == 0), stop=(dj == ND - 1))
                nc.vector.tensor_copy(V[:, ti, :], vp[:])

            # qT per head [dk, s-half]
            for h in range(H):
                qp = pps.tile([DK, SH], f32, tag="qp")
                for dj in range(ND):
                    nc.tensor.matmul(
                        qp[:], qw[:, dj, h * DK:(h + 1) * DK],
                        h1t[:, dj, 0:SH],
                        start=(dj == 0), stop=(dj == ND - 1))
                if trivial:
                    nc.vector.tensor_copy(QT[0:DK, h, :], qp[:])
                else:
                    nc.scalar.activation(QT[0:DK, h, :], qp[:], AF.Identity,
                                         bias=bqt[:, h:h + 1])

            # kT per head [dk, s-full]
            for h in range(H):
                kp = pps.tile([DK, S], f32, tag="kp")
                for dj in range(ND):
                    for n in range(2):
                        nc.tensor.matmul(
                            kp[:, n * SH:(n + 1) * SH],
                            kw[:, dj, h * DK:(h + 1) * DK],
                            h1t[:, dj, n * SH:(n + 1) * SH],
                            start=(dj == 0), stop=(dj == ND - 1))
                if trivial:
                    nc.scalar.copy(KT[0:DK, h, :], kp[:])
                else:
                    nc.scalar.activation(KT[0:DK, h, :], kp[:], AF.Identity,
                                         bias=bkt[:, h:h + 1])

        # ---- phase D: attention, software-pipelined per head-pair ----
        # D1(pair p): scores-nat -> exp(+rowsum) -> wt out, 1/Z column
        # D3(pair p-1): scores-T -> exp -> wt^T v -> normalize -> catT
        rz_all = stats.tile([P, H * NSH], f32, tag="rz")
        rzt = stats.tile([1, H, NSH, P], f32, tag="rzt")
        CATT = big.tile([P, NHV, SH], mdt)

        with tc.tile_pool(name="sc_ps", bufs=2, space="PSUM") as scps, \
             tc.tile_pool(name="ot_ps", bufs=3, space="PSUM") as otps, \
             tc.tile_pool(name="ewt", bufs=3) as ewtp, \
             tc.tile_pool(name="wtp", bufs=3) as wtp, \
             tc.tile_pool(name="ewtt", bufs=3) as ewttp, \
             tc.tile_pool(name="rzbp", bufs=2) as rzbp:

            def emit_d1(hp):
                for u in range(2):
                    h = 2 * hp + u
                    for si in range(NSH):
                        sc = scps.tile([P, S], f32, tag="sc")
                        for n in range(2):
                            nc.tensor.matmul(
                                sc[:, n * SH:(n + 1) * SH],
                                QT[:, h, si * P:(si + 1) * P],
                                KT[:, h, n * SH:(n + 1) * SH],
                                start=True, stop=True)
                        ewt = ewtp.tile([P, S], f32, tag="ewt")
                        z = stats.tile([P, 1], f32, tag="z")
                        nc.scalar.activation(ewt[:], sc[:], AF.Exp,
                                             accum_out=z[:])
                        col = h * NSH + si
                        nc.vector.reciprocal(rz_all[:, col:col + 1], z[:])
                        wtt = wtp.tile([P, S], f32, tag="wtt")
                        nc.vector.tensor_scalar_mul(
                            wtt[:], ewt[:], rz_all[:, col:col + 1])
                        nc.sync.dma_start(
                            wt_d[h].rearrange("(c p) t -> p c t", p=P)[:, si, :],
                            wtt[:])
                # bounce this pair's 1/Z columns into row form for D3
                c0 = hp * 2 * NSH
                nc.sync.dma_start(rz_dram[:, c0:c0 + 2 * NSH],
                                  rz_all[:, c0:c0 + 2 * NSH])
                nc.sync.dma_start(
                    rzt[:, 2 * hp:2 * hp + 2, :, :],
                    rz_dram.rearrange("p (h c) -> h c p", h=H)[None,
                                                              2 * hp:2 * hp + 2])

            def emit_d3(hp):
                ot0 = otps.tile([DV, SH], f32, tag="ot")
                ot1 = otps.tile([DV, SH], f32, tag="ot")
                ots = [ot0, ot1]
                for ti in range(NS):
                    sct = scps.tile([P, 2, SH], f32, tag="sc")
                    for u in range(2):
                        h = 2 * hp + u
                        nc.tensor.matmul(
                            sct[:, u, :], KT[:, h, ti * P:(ti + 1) * P],
                            QT[:, h, :], start=True, stop=True)
                    ewtt = ewttp.tile([P, 2, SH], mdt, tag="ewtt")
                    nc.scalar.activation(ewtt[:], sct[:], AF.Exp)
                    for u in range(2):
                        h = 2 * hp + u
                        nc.tensor.matmul(
                            ots[u][:], V[:, ti, h * DV:(h + 1) * DV],
                            ewtt[:, u, :],
                            start=(ti == 0), stop=(ti == NS - 1))
                for u in range(2):
                    h = 2 * hp + u
                    # broadcast 1/Z rows across the dv partitions via rank-1
                    rzb = otps.tile([DV, SH], f32, tag="ot")
                    nc.tensor.matmul(rzb[:], one_row[0:1, 0:DV],
                                     rzt[0:1, h, :, :], start=True, stop=True)
                    rzbs = rzbp.tile([DV, SH], f32, tag="rzbs")
                    nc.vector.tensor_copy(rzbs[:], rzb[:])
                    nc.vector.tensor_mul(
                        CATT[(h % 2) * DV:(h % 2) * DV + DV, h // 2, :],
                        ots[u][:], rzbs[:])

            for p in range(H // 2 + 1):
                if p < H // 2:
                    emit_d1(p)
                if p >= 1:
                    emit_d3(p - 1)

        # ---- wo projection + residual -> x2 ----
        X2 = big.tile([P, NSH, DM], f32)
        with tc.tile_pool(name="wo_pool", bufs=1) as wop, \
             tc.tile_pool(name="a_ps", bufs=2, space="PSUM") as aps:
            wo = wop.tile([P, NHV, DM], mdt, tag="wo")
            nc.sync.dma_start(wo[:], wo_d.rearrange("(j p) f -> p j f", p=P))
            if not trivial:
                worow = wop.tile([1, DM], f32, tag="worow")
                nc.sync.dma_start(worow[:], worow_d[:, :])
            for si in range(NSH):
                ap_ = aps.tile([P, DM], f32, tag="a")
                for kc in range(NHV):
                    nc.tensor.matmul(
                        ap_[:], CATT[:, kc, si * P:(si + 1) * P], wo[:, kc, :],
                        start=(kc == 0),
                        stop=(trivial and kc == NHV - 1))
                if not trivial:
                    nc.tensor.matmul(ap_[:], one_row[0:1, 0:P], worow[:],
                                     start=False, stop=True)
                nc.vector.tensor_add(X2[:, si, :], ap_[:], X[:, si, :])

        # ---- phase E: LN2 + FFN (streamed over df chunks) + residual ----
        h2t = big.tile([P, ND, SH], mdt)
        with tc.tile_pool(name="ln2", bufs=3) as ln2_pool, \
             tc.tile_pool(name="tp2_ps", bufs=2, space="PSUM") as tp2_ps:
            layernorm_t(X2, NSH, h2t, "b", ln2_pool, tp2_ps)

        with tc.tile_pool(name="ffn_c", bufs=1) as fcp, \
             tc.tile_pool(name="ffn_stream", bufs=3) as fsp, \
             tc.tile_pool(name="f2_ps", bufs=4, space="PSUM") as f2ps, \
             tc.tile_pool(name="f1_ps", bufs=2, space="PSUM") as f1ps:
            if not trivial:
                bfa = fcp.tile([P, NF], f32, tag="bfa")
                ffbrow = fcp.tile([1, DM], f32, tag="ffbrow")
                nc.sync.dma_start(bfa[:], bfa_d.rearrange("(j p) -> p j", p=P))
                nc.sync.dma_start(ffbrow[:], ffbrow_d[:, :])

            f2p0 = f2ps.tile([P, DM], f32, tag="f2")
            f2p1 = f2ps.tile([P, DM], f32, tag="f2")
            f2p2 = f2ps.tile([P, DM], f32, tag="f2")
            f2p3 = f2ps.tile([P, DM], f32, tag="f2")
            f2ps_t = [f2p0, f2p1, f2p2, f2p3]

            ffa_rr = ffa_d.rearrange("(j p) f -> p j f", p=P)
            ffb_rr = ffb_d.rearrange("(j p) f -> p j f", p=P)
            for fj in range(NF):
                ffa_fj = fsp.tile([P, ND, P], mdt, tag="ffa_fj")
                nc.sync.dma_start(ffa_fj[:], ffa_rr[:, :, fj * P:(fj + 1) * P])
                ffb_fj = fsp.tile([P, DM], mdt, tag="ffb_fj")
                nc.sync.dma_start(ffb_fj[:], ffb_rr[:, fj, :])
                fp_ = f1ps.tile([P, SH], f32, tag="f1")
                for dj in range(ND):
                    nc.tensor.matmul(
                        fp_[:], ffa_fj[:, dj, :], h2t[:, dj, :],
                        start=(dj == 0), stop=(dj == ND - 1))
                f1t_fj = fsp.tile([P, SH], mdt, tag="f1t_fj")
                if trivial:
                    nc.scalar.activation(f1t_fj[:], fp_[:], AF.Relu)
                else:
                    nc.scalar.activation(f1t_fj[:], fp_[:], AF.Relu,
                                         bias=bfa[:, fj:fj + 1])
                for si in range(NSH):
                    nc.tensor.matmul(
                        f2ps_t[si][:], f1t_fj[:, si * P:(si + 1) * P],
                        ffb_fj[:],
                        start=(fj == 0),
                        stop=(trivial and fj == NF - 1))

            for si in range(NSH):
                if not trivial:
                    nc.tensor.matmul(f2ps_t[si][:], one_row[0:1, 0:P],
                                     ffbrow[:], start=False, stop=True)
                nc.vector.tensor_add(X2[:, si, :], f2ps_t[si][:], X2[:, si, :])
                nc.sync.dma_start(
                    xo_d.rearrange("(c p) d -> p c d", p=P)[:, si, :],
                    X2[:, si, :])

    nc.compile()
    return nc


NSH_G = SH // P


def _get_program(mm_mode, trivial):
    key = (mm_mode, trivial)
    if key not in _prog_cache:
        _prog_cache[key] = _build_program(mm_mode, trivial)
    return _prog_cache[key]


def make_in_maps(x, mk, ln1_g, ln1_b, ln2_g, ln2_b, wq_w, wq_b, wk_w, wk_b,
                 wv_w, wv_b, wo_w, wo_b, ffa_w, ffa_b, ffb_w, ffb_b):
    x = np.asarray(x, dtype=np.float32)
    mk = np.asarray(mk)
    f = lambda a: np.asarray(a, dtype=np.float32)
    ln1_g, ln1_b, ln2_g, ln2_b = f(ln1_g), f(ln1_b), f(ln2_g), f(ln2_b)
    wq_w, wq_b, wk_w, wk_b = f(wq_w), f(wq_b), f(wk_w), f(wk_b)
    wv_w, wv_b, wo_w, wo_b = f(wv_w), f(wv_b), f(wo_w), f(wo_b)
    ffa_w, ffa_b, ffb_w, ffb_b = f(ffa_w), f(ffa_b), f(ffb_w), f(ffb_b)

    scale = np.float32(1.0 / np.sqrt(DK).astype(np.float32))
    # fold LN1 gain/bias into the qkv projections; fold 1/sqrt(dk) into q
    qw = (ln1_g[:, None, None] * wq_w.transpose(1, 0, 2) * scale).reshape(DM, DM)
    kw = (ln1_g[:, None, None] * wk_w.transpose(1, 0, 2)).reshape(DM, DM)
    vw = (ln1_g[:, None, None] * wv_w.transpose(1, 0, 2)).reshape(DM, DM)
    bq = np.einsum('d,hdk->hk', ln1_b, wq_w) * scale + wq_b * scale
    bk = np.einsum('d,hdk->hk', ln1_b, wk_w) + wk_b
    bv = np.einsum('d,hdv->hv', ln1_b, wv_w) + wv_b
    # v bias rides through softmax (rows sum to 1) into the wo bias row
    worow = (wo_b + bv.reshape(H * DV) @ wo_w)[None, :]
    # fold LN2 gain/bias into the FFN
    ffa = ln2_g[:, None] * ffa_w
    bfa = ln2_b @ ffa_w + ffa_b
    ffbrow = ffb_b[None, :]
    madd = np.where(mk, np.float32(0.0), np.float32(-1e9)).astype(np.float32)

    trivial = (mk.all() and not bq.any() and not bk.any() and not worow.any()
               and not bfa.any() and not ffbrow.any())

    in_maps = []
    for c in range(NCORES):
        b, s0 = c // 2, (c % 2) * SH
        xb = np.roll(x[b], -s0, axis=0)
        m = {
            "xb": np.ascontiguousarray(xb),
            "qw": qw, "kw": kw, "vw": vw, "wo": wo_w,
            "ffa": ffa, "ffb": ffb_w,
        }
        if not trivial:
            mrow = np.roll(madd[b], -s0)
            m.update({
                "madd": np.ascontiguousarray(np.broadcast_to(mrow, (H, S))),
                "bqt": np.ascontiguousarray(bq.T),
                "bkt": np.ascontiguousarray(bk.T),
                "worow": worow, "bfa": bfa, "ffbrow": ffbrow,
            })
        in_maps.append(m)
    return in_maps, trivial


def kernel(**inputs):
    from concourse.bass_utils import run_bass_kernel_spmd

    in_maps, trivial = make_in_maps(**inputs)
    nc = _get_program(MM_MODE, trivial)
    res = run_bass_kernel_spmd(nc, in_maps, list(range(NCORES)))

    x_out = np.empty((B, S, DM), dtype=np.float32)
    wt = np.empty((B, H, S, S), dtype=np.float32)
    for c in range(NCORES):
        b, s0 = c // 2, (c % 2) * SH
        x_out[b, s0:s0 + SH] = res.results[c]["x_out"]
        wt[b, :, s0:s0 + SH, :] = np.roll(res.results[c]["wt_out"], s0, axis=-1)
    return x_out, wt


# revision 23
# speedup vs baseline: 1.1089x; 1.1089x over previous
"""Trainium2 Bass kernel for nn_EncoderBlock (dense transformer block).

Sharding: 8 cores = 4 batches x 2 query-row halves. Each core computes the
full K/V for its batch (duplicated across the pair) and the full pipeline for
its 512 query rows; no collectives. Per-core inputs are rolled along the
sequence axis so every core's query rows are local rows 0..511, letting all
cores run one SPMD program; the host un-rolls the wt columns on gather.

Attention computes the score matrix in BOTH orientations (q-stationary for
the softmax/wt output, k-stationary for the wt^T @ v contraction) which is
cheaper than transposing 4M score elements on-chip. The two streams are
software-pipelined per head-pair so the scalar engine (exp) and tensor
engine (matmuls) overlap and the PE stays HAM-warm.

Outputs: (x_out [4,1024,512], wt [4,8,1024,1024]) matching the reference.
"""

import numpy as np

B, S, DM, H, DK, DV, DF = 4, 1024, 512, 8, 64, 64, 2048
SH = S // 2          # query rows per core
P = 128
EPS = 1e-5
NCORES = 8

# matmul dtype config: "f32r" (fast, ~1e-4 rel err) or "f32" (exact, 4x slower)
MM_MODE = "f32r"

_prog_cache = {}


def _build_program(mm_mode, trivial):
    import concourse.mybir as mybir
    import concourse.tile as tile
    from concourse import bacc
    from concourse.masks import make_identity
    from contextlib import ExitStack

    dt = mybir.dt
    AF = mybir.ActivationFunctionType
    f32 = dt.float32
    mdt = dt.float32r if mm_mode == "f32r" else dt.float32
    KA = DK if trivial else DK + 1   # contraction depth incl. mask-aug row

    nc = bacc.Bacc(None, target_bir_lowering=False)

    # ---- DRAM I/O ----
    xb_d = nc.dram_tensor("xb", [S, DM], f32, kind="ExternalInput")
    qw_d = nc.dram_tensor("qw", [DM, DM], mdt, kind="ExternalInput")
    kw_d = nc.dram_tensor("kw", [DM, DM], mdt, kind="ExternalInput")
    vw_d = nc.dram_tensor("vw", [DM, DM], mdt, kind="ExternalInput")
    wo_d = nc.dram_tensor("wo", [DM, DM], mdt, kind="ExternalInput")
    ffa_d = nc.dram_tensor("ffa", [DM, DF], mdt, kind="ExternalInput")
    ffb_d = nc.dram_tensor("ffb", [DF, DM], mdt, kind="ExternalInput")
    if not trivial:
        madd_d = nc.dram_tensor("madd", [H, S], mdt, kind="ExternalInput")
        bqt_d = nc.dram_tensor("bqt", [DK, H], f32, kind="ExternalInput")
        bkt_d = nc.dram_tensor("bkt", [DK, H], f32, kind="ExternalInput")
        worow_d = nc.dram_tensor("worow", [1, DM], f32, kind="ExternalInput")
        bfa_d = nc.dram_tensor("bfa", [DF], f32, kind="ExternalInput")
        ffbrow_d = nc.dram_tensor("ffbrow", [1, DM], f32, kind="ExternalInput")
    wt_d = nc.dram_tensor("wt_out", [H, SH, S], f32, kind="ExternalOutput")
    xo_d = nc.dram_tensor("x_out", [SH, DM], f32, kind="ExternalOutput")
    rz_dram = nc.dram_tensor("rz_scratch", [P, 32], f32)

    NS = S // P    # 8 t-chunks
    NSH = SH // P  # 4 s-chunks (query half)
    ND = DM // P   # 4 d-chunks
    NF = DF // P   # 16 df-chunks
    NHV = H * DV // P  # 4 chunks of concatenated head outputs

    with tile.TileContext(nc) as tc, ExitStack() as ctx:
        const = ctx.enter_context(tc.tile_pool(name="const", bufs=1))
        big = ctx.enter_context(tc.tile_pool(name="big", bufs=1))
        stats = ctx.enter_context(tc.tile_pool(name="stats", bufs=2))

        idn = const.tile([P, P], f32)
        make_identity(nc, idn[:])
        one_row = const.tile([1, S], f32)
        nc.gpsimd.memset(one_row[:], 1.0)
        eps_t = const.tile([P, 1], f32)
        nc.gpsimd.memset(eps_t[:], EPS)

        # ---- phase A: input ----
        X = big.tile([P, NS, DM], f32)           # x rolled, s-chunk major
        nc.sync.dma_start(X[:], xb_d.rearrange("(c p) d -> p c d", p=P))

        def layernorm_t(src, nchunks, dest_t, tagp, ln_pool, tp_ps):
            """LN over the free dim of src chunks, writing transposed output."""
            for si in range(nchunks):
                st6 = stats.tile([P, 6], f32, tag="st6" + tagp)
                nc.vector.bn_stats(st6[:], src[:, si, :])
                st2 = stats.tile([P, 2], f32, tag="st2" + tagp)
                nc.vector.bn_aggr(st2[:], st6[:])
                std = stats.tile([P, 1], f32, tag="std" + tagp)
                nc.scalar.activation(std[:], st2[:, 1:2], AF.Sqrt, bias=eps_t[:])
                r = stats.tile([P, 1], f32, tag="r" + tagp)
                nc.vector.reciprocal(r[:], std[:])
                nmr = stats.tile([P, 1], f32, tag="nmr" + tagp)
                nc.vector.scalar_tensor_tensor(
                    out=nmr[:], in0=st2[:, 0:1], scalar=-1.0, in1=r[:],
                    op0=mybir.AluOpType.mult, op1=mybir.AluOpType.mult)
                hn = ln_pool.tile([P, DM], f32, tag="hn" + tagp)
                nc.scalar.activation(hn[:], src[:, si, :], AF.Identity,
                                     bias=nmr[:], scale=r[:])
                for dj in range(ND):
                    tp = tp_ps.tile([P, P], f32, tag="tp" + tagp)
                    nc.tensor.transpose(tp[:], hn[:, dj * P:(dj + 1) * P], idn[:])
                    nc.vector.tensor_copy(dest_t[:, dj, si * P:(si + 1) * P],
                                          tp[:])

        # ---- phase B: LN1 -> h1T ----
        h1t = big.tile([P, ND, S], mdt)          # [d-part, d-chunk, s]
        with tc.tile_pool(name="ln", bufs=3) as ln_pool, \
             tc.tile_pool(name="tp_ps", bufs=2, space="PSUM") as tp_ps:
            layernorm_t(X, NS, h1t, "a", ln_pool, tp_ps)

        # ---- phase C: projections qT, kT, v ----
        if trivial:
            # head pairs stacked on partitions 0-63 / 64-127 so the K=64
            # score matmuls can row/col-pack two heads into the PE array
            QT = big.tile([P, H // 2, SH], mdt)
            KT = big.tile([P, H // 2, S], mdt)
        else:
            QT = big.tile([KA, H, SH], mdt)
            KT = big.tile([KA, H, S], mdt)
        V = big.tile([P, NS, DM], mdt)           # [t-part, t-chunk, (h dv)]
        if not trivial:
            nc.gpsimd.memset(QT[DK:DK + 1, :, :].bitcast(f32), 1.0)
            nc.sync.dma_start(KT[DK:DK + 1, :, :], madd_d[None, :, :])

        def qt_ap(h, sl):
            if trivial:
                return QT[(h % 2) * DK:(h % 2) * DK + DK, h // 2, sl]
            return QT[:, h, sl]

        def kt_ap(h, sl):
            if trivial:
                return KT[(h % 2) * DK:(h % 2) * DK + DK, h // 2, sl]
            return KT[:, h, sl]

        with tc.tile_pool(name="wpool", bufs=1) as wp, \
             tc.tile_pool(name="proj_ps", bufs=2, space="PSUM") as pps:
            qw = wp.tile([P, ND, DM], mdt, tag="qw")
            kw = wp.tile([P, ND, DM], mdt, tag="kw")
            vw = wp.tile([P, ND, DM], mdt, tag="vw")
            nc.sync.dma_start(qw[:], qw_d.rearrange("(j p) f -> p j f", p=P))
            nc.sync.dma_start(kw[:], kw_d.rearrange("(j p) f -> p j f", p=P))
            nc.sync.dma_start(vw[:], vw_d.rearrange("(j p) f -> p j f", p=P))
            if not trivial:
                bqt = wp.tile([DK, H], f32, tag="bqt")
                bkt = wp.tile([DK, H], f32, tag="bkt")
                nc.sync.dma_start(bqt[:], bqt_d[:, :])
                nc.sync.dma_start(bkt[:], bkt_d[:, :])

            # v (natural layout, all heads wide)
            for ti in range(NS):
                vp = pps.tile([P, DM], f32, tag="vp")
                for dj in range(ND):
                    nc.tensor.matmul(
                        vp[:], h1t[:, dj, ti * P:(ti + 1) * P], vw[:, dj, :],
                        start=(dj == 0), stop=(dj == ND - 1))
                nc.vector.tensor_copy(V[:, ti, :], vp[:])

            # qT per head [dk, s-half]
            for h in range(H):
                qp = pps.tile([DK, SH], f32, tag="qp")
                for dj in range(ND):
                    nc.tensor.matmul(
                        qp[:], qw[:, dj, h * DK:(h + 1) * DK],
                        h1t[:, dj, 0:SH],
                        start=(dj == 0), stop=(dj == ND - 1))
                if trivial:
                    nc.vector.tensor_copy(qt_ap(h, slice(None)), qp[:])
                else:
                    nc.scalar.activation(QT[0:DK, h, :], qp[:], AF.Identity,
                                         bias=bqt[:, h:h + 1])

            # kT per head [dk, s-full]
            for h in range(H):
                kp = pps.tile([DK, S], f32, tag="kp")
                for dj in range(ND):
                    for n in range(2):
                        nc.tensor.matmul(
                            kp[:, n * SH:(n + 1) * SH],
                            kw[:, dj, h * DK:(h + 1) * DK],
                            h1t[:, dj, n * SH:(n + 1) * SH],
                            start=(dj == 0), stop=(dj == ND - 1))
                if trivial:
                    nc.scalar.copy(kt_ap(h, slice(None)), kp[:])
                else:
                    nc.scalar.activation(KT[0:DK, h, :], kp[:], AF.Identity,
                                         bias=bkt[:, h:h + 1])

        # ---- phase D: attention, software-pipelined per head-pair ----
        # D1(pair p): scores-nat -> exp(+rowsum) -> wt out, 1/Z column
        # D3(pair p-1): scores-T -> exp -> wt^T v -> normalize -> catT
        rz_all = stats.tile([P, H * NSH], f32, tag="rz")
        rzt = stats.tile([1, H, NSH, P], f32, tag="rzt")
        CATT = big.tile([P, NHV, SH], mdt)

        with tc.tile_pool(name="sc_ps", bufs=2, space="PSUM") as scps, \
             tc.tile_pool(name="ot_ps", bufs=3, space="PSUM") as otps, \
             tc.tile_pool(name="ewt", bufs=3) as ewtp, \
             tc.tile_pool(name="wtp", bufs=3) as wtp, \
             tc.tile_pool(name="ewtt", bufs=3) as ewttp, \
             tc.tile_pool(name="rzbp", bufs=2) as rzbp:

            def softmax_out(h, si, sc):
                ewt = ewtp.tile([P, S], f32, tag="ewt")
                z = stats.tile([P, 1], f32, tag="z")
                nc.scalar.activation(ewt[:], sc[:], AF.Exp, accum_out=z[:])
                col = h * NSH + si
                nc.vector.reciprocal(rz_all[:, col:col + 1], z[:])
                wtt = wtp.tile([P, S], f32, tag="wtt")
                nc.vector.tensor_scalar_mul(
                    wtt[:], ewt[:], rz_all[:, col:col + 1])
                nc.sync.dma_start(
                    wt_d[h].rearrange("(c p) t -> p c t", p=P)[:, si, :],
                    wtt[:])

            def emit_d1(hp):
                if trivial:
                    for si in range(NSH):
                        sc0 = scps.tile([P, S], f32, tag="sc")
                        sc1 = scps.tile([P, S], f32, tag="sc")
                        scs = [sc0, sc1]
                        for n in range(2):
                            for u in range(2):
                                h = 2 * hp + u
                                nc.tensor.matmul(
                                    scs[u][:, n * SH:(n + 1) * SH],
                                    qt_ap(h, slice(si * P, (si + 1) * P)),
                                    kt_ap(h, slice(n * SH, (n + 1) * SH)),
                                    start=True, stop=True,
                                    tile_position=(u * DK, 0))
                        for u in range(2):
                            softmax_out(2 * hp + u, si, scs[u])
                else:
                    for u in range(2):
                        h = 2 * hp + u
                        for si in range(NSH):
                            sc = scps.tile([P, S], f32, tag="sc")
                            for n in range(2):
                                nc.tensor.matmul(
                                    sc[:, n * SH:(n + 1) * SH],
                                    QT[:, h, si * P:(si + 1) * P],
                                    KT[:, h, n * SH:(n + 1) * SH],
                                    start=True, stop=True)
                            softmax_out(h, si, sc)
                # bounce this pair's 1/Z columns into row form for D3
                c0 = hp * 2 * NSH
                nc.sync.dma_start(rz_dram[:, c0:c0 + 2 * NSH],
                                  rz_all[:, c0:c0 + 2 * NSH])
                nc.sync.dma_start(
                    rzt[:, 2 * hp:2 * hp + 2, :, :],
                    rz_dram.rearrange("p (h c) -> h c p", h=H)[None,
                                                              2 * hp:2 * hp + 2])

            def emit_d3(hp):
                if True:
                    ot0 = otps.tile([DV, SH], f32, tag="ot")
                    ot1 = otps.tile([DV, SH], f32, tag="ot")
                    ots = [ot0, ot1]
                    for ti in range(NS):
                        sct = scps.tile([P, 2, SH], f32, tag="sc")
                        for u in range(2):
                            h = 2 * hp + u
                            if trivial:
                                nc.tensor.matmul(
                                    sct[:, u, :],
                                    kt_ap(h, slice(ti * P, (ti + 1) * P)),
                                    qt_ap(h, slice(None)),
                                    start=True, stop=True,
                                    tile_position=(u * DK, 0))
                            else:
                                nc.tensor.matmul(
                                    sct[:, u, :], KT[:, h, ti * P:(ti + 1) * P],
                                    QT[:, h, :], start=True, stop=True)
                        ewtt = ewttp.tile([P, 2, SH], mdt, tag="ewtt")
                        nc.scalar.activation(ewtt[:], sct[:], AF.Exp)
                        for u in range(2):
                            h = 2 * hp + u
                            nc.tensor.matmul(
                                ots[u][:], V[:, ti, h * DV:(h + 1) * DV],
                                ewtt[:, u, :],
                                start=(ti == 0), stop=(ti == NS - 1))
                    for u in range(2):
                        h = 2 * hp + u
                        rzb = otps.tile([DV, SH], f32, tag="ot")
                        nc.tensor.matmul(rzb[:], one_row[0:1, 0:DV],
                                         rzt[0:1, h, :, :],
                                         start=True, stop=True)
                        rzbs = rzbp.tile([DV, SH], f32, tag="rzbs")
                        nc.vector.tensor_copy(rzbs[:], rzb[:])
                        nc.vector.tensor_mul(
                            CATT[(h % 2) * DV:(h % 2) * DV + DV, h // 2, :],
                            ots[u][:], rzbs[:])

            for p in range(H // 2 + 1):
                if p < H // 2:
                    emit_d1(p)
                if p >= 1:
                    emit_d3(p - 1)

        # ---- wo projection + residual -> x2 ----
        X2 = big.tile([P, NSH, DM], f32)
        with tc.tile_pool(name="wo_pool", bufs=1) as wop, \
             tc.tile_pool(name="a_ps", bufs=2, space="PSUM") as aps:
            wo = wop.tile([P, NHV, DM], mdt, tag="wo")
            nc.sync.dma_start(wo[:], wo_d.rearrange("(j p) f -> p j f", p=P))
            if not trivial:
                worow = wop.tile([1, DM], f32, tag="worow")
                nc.sync.dma_start(worow[:], worow_d[:, :])
            for si in range(NSH):
                ap_ = aps.tile([P, DM], f32, tag="a")
                for kc in range(NHV):
                    nc.tensor.matmul(
                        ap_[:], CATT[:, kc, si * P:(si + 1) * P], wo[:, kc, :],
                        start=(kc == 0),
                        stop=(trivial and kc == NHV - 1))
                if not trivial:
                    nc.tensor.matmul(ap_[:], one_row[0:1, 0:P], worow[:],
                                     start=False, stop=True)
                nc.vector.tensor_add(X2[:, si, :], ap_[:], X[:, si, :])

        # ---- phase E: LN2 + FFN (streamed over df chunks) + residual ----
        h2t = big.tile([P, ND, SH], mdt)
        with tc.tile_pool(name="ln2", bufs=3) as ln2_pool, \
             tc.tile_pool(name="tp2_ps", bufs=2, space="PSUM") as tp2_ps:
            layernorm_t(X2, NSH, h2t, "b", ln2_pool, tp2_ps)

        with tc.tile_pool(name="ffn_c", bufs=1) as fcp, \
             tc.tile_pool(name="ffn_stream", bufs=3) as fsp, \
             tc.tile_pool(name="f2_ps", bufs=4, space="PSUM") as f2ps, \
             tc.tile_pool(name="f1_ps", bufs=2, space="PSUM") as f1ps:
            if not trivial:
                bfa = fcp.tile([P, NF], f32, tag="bfa")
                ffbrow = fcp.tile([1, DM], f32, tag="ffbrow")
                nc.sync.dma_start(bfa[:], bfa_d.rearrange("(j p) -> p j", p=P))
                nc.sync.dma_start(ffbrow[:], ffbrow_d[:, :])

            f2p0 = f2ps.tile([P, DM], f32, tag="f2")
            f2p1 = f2ps.tile([P, DM], f32, tag="f2")
            f2p2 = f2ps.tile([P, DM], f32, tag="f2")
            f2p3 = f2ps.tile([P, DM], f32, tag="f2")
            f2ps_t = [f2p0, f2p1, f2p2, f2p3]

            ffa_rr = ffa_d.rearrange("(j p) f -> p j f", p=P)
            ffb_rr = ffb_d.rearrange("(j p) f -> p j f", p=P)
            for fj in range(NF):
                ffa_fj = fsp.tile([P, ND, P], mdt, tag="ffa_fj")
                nc.sync.dma_start(ffa_fj[:], ffa_rr[:, :, fj * P:(fj + 1) * P])
                ffb_fj = fsp.tile([P, DM], mdt, tag="ffb_fj")
                nc.sync.dma_start(ffb_fj[:], ffb_rr[:, fj, :])
                fp_ = f1ps.tile([P, SH], f32, tag="f1")
                for dj in range(ND):
                    nc.tensor.matmul(
                        fp_[:], ffa_fj[:, dj, :], h2t[:, dj, :],
                        start=(dj == 0), stop=(dj == ND - 1))
                f1t_fj = fsp.tile([P, SH], mdt, tag="f1t_fj")
                if trivial:
                    nc.scalar.activation(f1t_fj[:], fp_[:], AF.Relu)
                else:
                    nc.scalar.activation(f1t_fj[:], fp_[:], AF.Relu,
                                         bias=bfa[:, fj:fj + 1])
                for si in range(NSH):
                    nc.tensor.matmul(
                        f2ps_t[si][:], f1t_fj[:, si * P:(si + 1) * P],
                        ffb_fj[:],
                        start=(fj == 0),
                        stop=(trivial and fj == NF - 1))

            for si in range(NSH):
                if not trivial:
                    nc.tensor.matmul(f2ps_t[si][:], one_row[0:1, 0:P],
                                     ffbrow[:], start=False, stop=True)
                nc.vector.tensor_add(X2[:, si, :], f2ps_t[si][:], X2[:, si, :])
                nc.sync.dma_start(
                    xo_d.rearrange("(c p) d -> p c d", p=P)[:, si, :],
                    X2[:, si, :])

    nc.compile()
    return nc


NSH_G = SH // P


def _get_program(mm_mode, trivial):
    key = (mm_mode, trivial)
    if key not in _prog_cache:
        _prog_cache[key] = _build_program(mm_mode, trivial)
    return _prog_cache[key]


def make_in_maps(x, mk, ln1_g, ln1_b, ln2_g, ln2_b, wq_w, wq_b, wk_w, wk_b,
                 wv_w, wv_b, wo_w, wo_b, ffa_w, ffa_b, ffb_w, ffb_b):
    x = np.asarray(x, dtype=np.float32)
    mk = np.asarray(mk)
    f = lambda a: np.asarray(a, dtype=np.float32)
    ln1_g, ln1_b, ln2_g, ln2_b = f(ln1_g), f(ln1_b), f(ln2_g), f(ln2_b)
    wq_w, wq_b, wk_w, wk_b = f(wq_w), f(wq_b), f(wk_w), f(wk_b)
    wv_w, wv_b, wo_w, wo_b = f(wv_w), f(wv_b), f(wo_w), f(wo_b)
    ffa_w, ffa_b, ffb_w, ffb_b = f(ffa_w), f(ffa_b), f(ffb_w), f(ffb_b)

    scale = np.float32(1.0 / np.sqrt(DK).astype(np.float32))
    # fold LN1 gain/bias into the qkv projections; fold 1/sqrt(dk) into q
    qw = (ln1_g[:, None, None] * wq_w.transpose(1, 0, 2) * scale).reshape(DM, DM)
    kw = (ln1_g[:, None, None] * wk_w.transpose(1, 0, 2)).reshape(DM, DM)
    vw = (ln1_g[:, None, None] * wv_w.transpose(1, 0, 2)).reshape(DM, DM)
    bq = np.einsum('d,hdk->hk', ln1_b, wq_w) * scale + wq_b * scale
    bk = np.einsum('d,hdk->hk', ln1_b, wk_w) + wk_b
    bv = np.einsum('d,hdv->hv', ln1_b, wv_w) + wv_b
    # v bias rides through softmax (rows sum to 1) into the wo bias row
    worow = (wo_b + bv.reshape(H * DV) @ wo_w)[None, :]
    # fold LN2 gain/bias into the FFN
    ffa = ln2_g[:, None] * ffa_w
    bfa = ln2_b @ ffa_w + ffa_b
    ffbrow = ffb_b[None, :]
    madd = np.where(mk, np.float32(0.0), np.float32(-1e9)).astype(np.float32)

    trivial = (mk.all() and not bq.any() and not bk.any() and not worow.any()
               and not bfa.any() and not ffbrow.any())

    in_maps = []
    for c in range(NCORES):
        b, s0 = c // 2, (c % 2) * SH
        xb = np.roll(x[b], -s0, axis=0)
        m = {
            "xb": np.ascontiguousarray(xb),
            "qw": qw, "kw": kw, "vw": vw, "wo": wo_w,
            "ffa": ffa, "ffb": ffb_w,
        }
        if not trivial:
            mrow = np.roll(madd[b], -s0)
            m.update({
                "madd": np.ascontiguousarray(np.broadcast_to(mrow, (H, S))),
                "bqt": np.ascontiguousarray(bq.T),
                "bkt": np.ascontiguousarray(bk.T),
                "worow": worow, "bfa": bfa, "ffbrow": ffbrow,
            })
        in_maps.append(m)
    return in_maps, trivial


def kernel(**inputs):
    from concourse.bass_utils import run_bass_kernel_spmd

    in_maps, trivial = make_in_maps(**inputs)
    nc = _get_program(MM_MODE, trivial)
    res = run_bass_kernel_spmd(nc, in_maps, list(range(NCORES)))

    x_out = np.empty((B, S, DM), dtype=np.float32)
    wt = np.empty((B, H, S, S), dtype=np.float32)
    for c in range(NCORES):
        b, s0 = c // 2, (c % 2) * SH
        x_out[b, s0:s0 + SH] = res.results[c]["x_out"]
        wt[b, :, s0:s0 + SH, :] = np.roll(res.results[c]["wt_out"], s0, axis=-1)
    return x_out, wt
